# revision 13
# baseline (speedup 1.0000x reference)
"""TRN2 Bass kernel for nn_CommLayer (gnn message passing).

Math: x [B=65536, 512] viewed as [B, 8 agents, 64]; per agent a:
    y_a = tanh(x_a @ Wh.T + (sum_{a'!=a} x_{a'}) @ Wc.T / 7)
Rewritten with s = sum_a x_a:
    y_a = tanh(x_a @ WdT + s @ Wc7T),  WdT = Wh.T - Wc.T/7, Wc7T = Wc.T/7
a block-diagonal matmul plus a shared rank-64 term -- 7x less PE work
than the dense 512x512 matmul. Data-parallel across the 8 cores (8192
batch rows each); the two [64, 64] weights are replicated.

Everything runs in the TRANSPOSED domain in fp16 (rel err 8.8e-3 vs
2e-2 budget); tanh output ships as int8 (x127), halving store traffic.
Per-core HBM: 9.4 MB loads + 4.2 MB stores = 13.6 MB ~= 34 us at the
~400 GB/s the 16 SDMA engines sustain -- the roofline. PE (28 us),
ACT tanh (31 us), DVE quant (20 us) fit underneath, so the wins are
all DMA-pipeline shaping.

Layout trick: the wcs stationary only has weights in 64 of its 128
rows (the other moving partitions multiply zeros), so ONE [128, 512]
s-pack block per group carries s.T[:, 0:512] in partitions 0:64 and
s.T[:, 512:1024] in partitions 64:128, selected by two stationaries
(wcs_lo / wcs_hi). Full k=128 matmuls (the PE HAM clock gate throttles
half-array work), zero on-device s marshalling, zero duplicate bytes.

The semaphore program is written by hand on raw bass (no TileContext):
the Tile scheduler's conservative cross-engine semaphore targets
repeatedly serialized stores behind unrelated loads in traced
iterations. Hard-won rules baked in below:
  - a dma_start BLOCKS its issuing sequencer while the HWDGE ring is
    full (~6 entries): all loads live on the otherwise-idle sync
    engine, never on an engine with compute.
  - stores are FIFO-deferred behind all loads on the same qSP ring, so
    loads run the SDMA engines at full rate and the ACT tanh cadence
    (the steady-state critical engine) is never load-starved; store
    dispatches wait on quant sems the SP sequencer eats while idle.
  - one semaphore per load DMA: a single counting sem across
    concurrently-draining DMAs is UNSAFE (the 16 per-engine
    completion incs interleave across DMAs).
  - HAM warmup matmuls on a memset tile ramp the PE clock gate
    1.2 -> 2.4 GHz during the initial load latency; a >3.4 us PE idle
    gap mid-kernel re-throttles and the 2x slowdown cascades.
  - og x4 / oq x8 buffers decouple quant -> tanh -> PE from store
    completion jitter.

Pipeline (j = 0..15 half-groups, production order g=0..7, half=1 then 0):
  sync  : 11 load DMAs (no waits) then 16 store DMAs (wait quant j)
  tensor: 10 HAM-warmup matmuls, then per half: wait load sem, wait
          tanh j-2 (psum buf recycle), 8 matmuls, inc ps
  scalar: per half: wait ps, wait quant j-4 (og recycle), tanh, inc tg
  vector: memset; per half: wait tg, wait store j-8 done (oq recycle),
          quant, inc qd
  gpsimd: idle until the end: waits final ld/sd values, then the
          cleanup barrier + semaphore clears run.
"""
import sys

sys.path.insert(0, "/opt/trn_rl_repo")

import numpy as np

BATCH = 65536
D = 512
NAGENT = 8
DA = 64
NORM = NAGENT - 1
NCORES = 8
SHARD = BATCH // NCORES  # 8192
R = 1024                 # rows per group
NGROUP = SHARD // R      # 8
NCHUNK = D // 128        # 4
XCOL = NCHUNK * R        # 4096
SCOL = 512
GCOL = XCOL + SCOL       # 4608

_CACHE: dict = {}


def _build_nc():
    import concourse.mybir as mybir
    from concourse import bacc
    from contextlib import ExitStack

    nc = bacc.Bacc("TRN2", target_bir_lowering=False, debug=False)

    f16 = mybir.dt.float16
    f32 = mybir.dt.float32
    i8 = mybir.dt.int8
    Tanh = mybir.ActivationFunctionType.Tanh

    x7_d = nc.dram_tensor(
        "x7", [NGROUP * 128, GCOL], f16, kind="ExternalInput"
    )
    wpk_d = nc.dram_tensor("wpk", [128, 384], f16, kind="ExternalInput")
    y4_d = nc.dram_tensor(
        "y4", [NGROUP * 128, XCOL], i8, kind="ExternalOutput"
    )

    xv = x7_d[:].rearrange("(g p) f -> g p f", p=128)
    yv = y4_d[:].rearrange("(g p) f -> g p f", p=128)

    with nc.cleanup_on_exit():
        with ExitStack() as ctx:
            # one sem per load DMA: a single counting sem across
            # concurrently-draining DMAs is unsafe (per-engine incs
            # interleave, so value 32 does not imply DMA #2 finished)
            lsem = [nc.alloc_semaphore(f"ld{i}") for i in range(10)]
            # store sems rotate over 8 lanes; lane reuse is gated by the
            # quant j -> store j dependency chain, so values are stable
            ssem = [nc.alloc_semaphore(f"sd{i}") for i in range(8)]
            md = nc.alloc_semaphore("md")   # memset done
            ps = nc.alloc_semaphore("ps")   # psum half ready (PE)
            tg = nc.alloc_semaphore("tg")   # tanh done (ACT)
            qd = nc.alloc_semaphore("qd")   # quant done (DVE)

            wpk = ctx.enter_context(nc.sbuf_tensor("wpk_s", [128, 384], f16))
            mset = ctx.enter_context(nc.sbuf_tensor("mset", [128, 512], f16))
            xgs = [
                ctx.enter_context(
                    nc.sbuf_tensor(f"xg{g}", [128, GCOL], f16)
                )
                for g in range(NGROUP)
            ]
            ogs = [
                ctx.enter_context(
                    nc.sbuf_tensor(f"og{b}", [128, 2048], f16)
                )
                for b in range(4)
            ]
            oqs = [
                ctx.enter_context(
                    nc.sbuf_tensor(f"oq{b}", [128, 2048], i8)
                )
                for b in range(8)
            ]
            psy = [
                ctx.enter_context(
                    nc.psum_tensor(f"psy{b}", [128, 2048], f32)
                )
                for b in range(2)
            ]

            wd2 = wpk[:, 0:128]
            wcs = (wpk[:, 128:256], wpk[:, 256:384])

            halves = [(g, h) for g in range(NGROUP) for h in (1, 0)]

            # ---- loads: wpk + g0A on gpsimd SWDGE (Q7 emission starts
            # ~1 us after main, beating the HWDGE ring's ~2.4 us spin-up
            # for the head of the pipeline); everything else on sync ----
            nc.gpsimd.memset(mset[:], 0.0).then_inc(md, 1)
            nc.gpsimd.dma_start(wpk[:], wpk_d[:]).then_inc(lsem[0], 16)
            nc.gpsimd.dma_start(
                xgs[0][:, 2048:GCOL], xv[0][:, 2048:GCOL]
            ).then_inc(lsem[1], 16)
            nc.sync.dma_start(
                xgs[0][:, 0:2048], xv[0][:, 0:2048]
            ).then_inc(lsem[2], 16)
            for g in range(1, NGROUP):
                nc.sync.dma_start(xgs[g][:], xv[g]).then_inc(lsem[2 + g], 16)
            for j, (g, half) in enumerate(halves):
                nc.sync.wait_ge(qd, j + 1)
                nc.sync.dma_start(
                    yv[g][:, half * 2048:(half + 1) * 2048], oqs[j % 8][:]
                ).then_inc(ssem[j % 8], 16)

            # ---- vector: quants (memset moved to gpsimd so PE warmups
            # start right at main) ----
            for j in range(16):
                nc.vector.wait_ge(tg, j + 1)
                if j >= 8:
                    nc.vector.wait_ge(ssem[j % 8], 16 * (j // 8))
                nc.vector.tensor_scalar_mul(
                    oqs[j % 8][:], ogs[j % 4][:], 127.0
                ).then_inc(qd, 1)

            # ---- tensor: warmups then the half pipeline ----
            nc.tensor.wait_ge(md, 1)
            for w in range(8):
                nc.tensor.matmul(
                    psy[0][:, 0:512], mset[:, 0:128], mset[:],
                    start=True, stop=True,
                )
            for j, (g, half) in enumerate(halves):
                if j == 0:
                    nc.tensor.wait_ge(lsem[0], 16)  # wpk
                    nc.tensor.wait_ge(lsem[1], 16)  # g0 s+chunks23
                elif j == 1:
                    nc.tensor.wait_ge(lsem[2], 16)  # g0 chunks01
                elif half == 1:
                    nc.tensor.wait_ge(lsem[2 + g], 16)
                if j >= 2:
                    nc.tensor.wait_ge(tg, j - 1)
                pb = psy[j % 2]
                xg = xgs[g]
                spk = xg[:, XCOL:GCOL]
                for ci in range(2):
                    co = 2 * half + ci
                    for h in range(2):
                        fs = slice(ci * R + h * 512, ci * R + (h + 1) * 512)
                        nc.tensor.matmul(
                            pb[:, fs], wcs[h], spk,
                            start=True, stop=False,
                        )
                        mm = nc.tensor.matmul(
                            pb[:, fs], wd2,
                            xg[:, co * R + h * 512:co * R + (h + 1) * 512],
                            start=False, stop=True,
                        )
                mm.then_inc(ps, 1)

            # ---- scalar: tanhs ----
            for j in range(16):
                nc.scalar.wait_ge(ps, j + 1)
                if j >= 4:
                    nc.scalar.wait_ge(qd, j - 3)
                nc.scalar.activation(
                    ogs[j % 4][:], psy[j % 2][:], Tanh
                ).then_inc(tg, 1)

            # ---- end: settle DMA sems before cleanup's clear ----
            for s in lsem:
                nc.gpsimd.wait_ge(s, 16)
            for s in ssem:
                nc.gpsimd.wait_ge(s, 32)
            nc.all_engine_barrier()

    nc.compile()
    return nc


def _get_nc():
    if "nc" not in _CACHE:
        _CACHE["nc"] = _build_nc()
    return _CACHE["nc"]


def _prepare_in_maps(inputs) -> list[dict]:
    """Full inputs -> per-core in_maps (host does transpose + fp16 cast)."""
    x = np.asarray(inputs["x"], dtype=np.float32)
    hw = np.asarray(inputs["hidden_weights"], dtype=np.float32)
    cw = np.asarray(inputs["communication_weights"], dtype=np.float32)
    assert x.shape == (BATCH, D), x.shape

    wc7t = cw.T / np.float32(NORM)          # [64, 64]
    wdt = hw.T - wc7t                       # [64, 64]
    wpk = np.zeros((128, 384), dtype=np.float16)
    wpk[0:64, 0:64] = wdt                   # wd2 block-diagonal
    wpk[64:128, 64:128] = wdt
    wpk[0:64, 128:192] = wc7t               # wcs_lo
    wpk[0:64, 192:256] = wc7t
    wpk[64:128, 256:320] = wc7t             # wcs_hi
    wpk[64:128, 320:384] = wc7t

    s = x.reshape(BATCH, NAGENT, DA).sum(axis=1)
    x16 = x.astype(np.float16)
    s16 = s.astype(np.float16)

    in_maps = []
    for i in range(NCORES):
        rows = slice(i * SHARD, (i + 1) * SHARD)
        xt = x16[rows].T
        st = s16[rows].T
        x7 = np.empty((NGROUP, 128, GCOL), dtype=np.float16)
        x7[:, :, 0:XCOL] = (
            xt.reshape(NCHUNK, 128, NGROUP, R).transpose(2, 1, 0, 3)
            .reshape(NGROUP, 128, XCOL)
        )
        x7[:, :, XCOL:GCOL] = (
            st.reshape(DA, NGROUP, 2, 512).transpose(1, 2, 0, 3)
            .reshape(NGROUP, 128, SCOL)
        )
        in_maps.append({"x7": x7.reshape(NGROUP * 128, GCOL), "wpk": wpk})
    return in_maps


def _decode_out(res) -> np.ndarray:
    y = np.empty((BATCH, D), dtype=np.float32)
    inv = np.float32(1.0 / 127.0)
    for i, r in enumerate(res.results):
        y4 = r["y4"].reshape(NGROUP, 128, NCHUNK, R)
        yi = y4.transpose(0, 3, 2, 1).reshape(SHARD, D)
        y[i * SHARD:(i + 1) * SHARD] = yi
    y *= inv
    return y


def kernel(**inputs) -> np.ndarray:
    from concourse.bass_utils import run_bass_kernel_spmd

    nc = _get_nc()
    in_maps = _prepare_in_maps(inputs)
    res = run_bass_kernel_spmd(nc, in_maps, core_ids=list(range(NCORES)))
    return _decode_out(res)


# revision 15
# speedup vs baseline: 1.1371x; 1.1371x over previous
"""TRN2 Bass kernel for nn_CommLayer -- raw-bass variant (no TileContext).

Same math/layout as kernel.py v8 (see its docstring): transposed-domain
fp16, one [128, 4608] tile per 1024-row group (x.T chunks + s-pack),
wcs_lo/wcs_hi stationaries select the s.T halves packed across the
moving partitions, int8 x127 output.

Difference: the pipeline's semaphore program is written by hand on raw
bass instead of TileContext. The Tile exit protocol (drain over a
27-proc vector clock + 2 all-engine barriers + semaphore range-clears)
costs a fixed ~8.6 us in every traced Tile version; the manual exit
(wait final DMA sems -> barrier -> clear -> barrier) is ~2 us. It also
removes the scheduler's conservative semaphore targets on the hot path.

Pipeline (j = 0..15 half-groups, production order g=0..7, half=1 then 0):
  sync  : 11 load DMAs (no waits) then 16 store DMAs (wait quant j)
  tensor: 8 HAM-warmup matmuls, then per half: wait load sem, wait
          tanh j-2 (psum buf recycle), 8 matmuls, inc ps
  scalar: per half: wait ps, wait quant j-4 (og recycle), tanh, inc tg
  vector: memset; per half: wait tg, wait store j-8 done (oq recycle),
          quant, inc qd
  gpsimd: idle until the end: waits final ld/sd values, then the
          cleanup barrier + semaphore clears run.
"""
import sys

sys.path.insert(0, "/opt/trn_rl_repo")

import numpy as np

BATCH = 65536
D = 512
NAGENT = 8
DA = 64
NORM = NAGENT - 1
NCORES = 8
SHARD = BATCH // NCORES  # 8192
R = 1024                 # rows per group
NGROUP = SHARD // R      # 8
NCHUNK = D // 128        # 4
XCOL = NCHUNK * R        # 4096
SCOL = 512
GCOL = XCOL + SCOL       # 4608

_CACHE: dict = {}


def _build_nc():
    import concourse.mybir as mybir
    from concourse import bacc
    from contextlib import ExitStack

    nc = bacc.Bacc("TRN2", target_bir_lowering=False, debug=False)

    f16 = mybir.dt.float16
    f32 = mybir.dt.float32
    i8 = mybir.dt.int8
    Tanh = mybir.ActivationFunctionType.Tanh

    x7_d = nc.dram_tensor(
        "x7", [NGROUP * 128, GCOL], f16, kind="ExternalInput"
    )
    wpk_d = nc.dram_tensor("wpk", [128, 384], f16, kind="ExternalInput")
    y4_d = nc.dram_tensor(
        "y4", [NGROUP * 128, XCOL], i8, kind="ExternalOutput"
    )

    xv = x7_d[:].rearrange("(g p) f -> g p f", p=128)
    yv = y4_d[:].rearrange("(g p) f -> g p f", p=128)

    with nc.cleanup_on_exit():
        with ExitStack() as ctx:
            # one sem per load DMA: a single counting sem across
            # concurrently-draining DMAs is unsafe (per-engine incs
            # interleave, so value 32 does not imply DMA #2 finished)
            lsem = [nc.alloc_semaphore(f"ld{i}") for i in range(10)]
            # store sems rotate over 8 lanes; lane reuse is gated by the
            # quant j -> store j dependency chain, so values are stable
            ssem = [nc.alloc_semaphore(f"sd{i}") for i in range(8)]
            md = nc.alloc_semaphore("md")   # memset done
            ps = nc.alloc_semaphore("ps")   # psum half ready (PE)
            tg = nc.alloc_semaphore("tg")   # tanh done (ACT)
            qd = nc.alloc_semaphore("qd")   # quant done (DVE)

            wpk = ctx.enter_context(nc.sbuf_tensor("wpk_s", [128, 384], f16))
            mset = ctx.enter_context(nc.sbuf_tensor("mset", [128, 512], f16))
            xgs = [
                ctx.enter_context(
                    nc.sbuf_tensor(f"xg{g}", [128, GCOL], f16)
                )
                for g in range(NGROUP)
            ]
            ogs = [
                ctx.enter_context(
                    nc.sbuf_tensor(f"og{b}", [128, 2048], f16)
                )
                for b in range(4)
            ]
            oqs = [
                ctx.enter_context(
                    nc.sbuf_tensor(f"oq{b}", [128, 2048], i8)
                )
                for b in range(8)
            ]
            psy = [
                ctx.enter_context(
                    nc.psum_tensor(f"psy{b}", [128, 2048], f32)
                )
                for b in range(2)
            ]

            wd2 = wpk[:, 0:128]
            wcs = (wpk[:, 128:256], wpk[:, 256:384])

            halves = [(g, h) for g in range(NGROUP) for h in (1, 0)]

            # ---- sync: loads then stores ----
            nc.sync.dma_start(wpk[:], wpk_d[:]).then_inc(lsem[0], 16)
            nc.sync.dma_start(
                xgs[0][:, 2048:GCOL], xv[0][:, 2048:GCOL]
            ).then_inc(lsem[1], 16)
            nc.sync.dma_start(
                xgs[0][:, 0:2048], xv[0][:, 0:2048]
            ).then_inc(lsem[2], 16)
            for g in range(1, NGROUP):
                nc.sync.dma_start(xgs[g][:], xv[g]).then_inc(lsem[2 + g], 16)
            for j, (g, half) in enumerate(halves):
                nc.sync.wait_ge(qd, j + 1)
                nc.sync.dma_start(
                    yv[g][:, half * 2048:(half + 1) * 2048], oqs[j % 8][:]
                ).then_inc(ssem[j % 8], 16)

            # ---- vector: memset then quants ----
            nc.vector.memset(mset[:], 0.0).then_inc(md, 1)
            for j in range(16):
                nc.vector.wait_ge(tg, j + 1)
                if j >= 8:
                    nc.vector.wait_ge(ssem[j % 8], 16 * (j // 8))
                nc.vector.tensor_scalar_mul(
                    oqs[j % 8][:], ogs[j % 4][:], 127.0
                ).then_inc(qd, 1)

            # ---- tensor: warmups then the half pipeline ----
            nc.tensor.wait_ge(md, 1)
            for w in range(8):
                nc.tensor.matmul(
                    psy[0][:, 0:512], mset[:, 0:128], mset[:],
                    start=True, stop=True,
                )
            for j, (g, half) in enumerate(halves):
                if j == 0:
                    nc.tensor.wait_ge(lsem[0], 16)  # wpk
                    nc.tensor.wait_ge(lsem[1], 16)  # g0 s+chunks23
                elif j == 1:
                    nc.tensor.wait_ge(lsem[2], 16)  # g0 chunks01
                elif half == 1:
                    nc.tensor.wait_ge(lsem[2 + g], 16)
                if j >= 2:
                    nc.tensor.wait_ge(tg, j - 1)
                pb = psy[j % 2]
                xg = xgs[g]
                spk = xg[:, XCOL:GCOL]
                for ci in range(2):
                    co = 2 * half + ci
                    for h in range(2):
                        fs = slice(ci * R + h * 512, ci * R + (h + 1) * 512)
                        nc.tensor.matmul(
                            pb[:, fs], wcs[h], spk,
                            start=True, stop=False,
                        )
                        mm = nc.tensor.matmul(
                            pb[:, fs], wd2,
                            xg[:, co * R + h * 512:co * R + (h + 1) * 512],
                            start=False, stop=True,
                        )
                mm.then_inc(ps, 1)

            # ---- scalar: tanhs ----
            for j in range(16):
                nc.scalar.wait_ge(ps, j + 1)
                if j >= 4:
                    nc.scalar.wait_ge(qd, j - 3)
                nc.scalar.activation(
                    ogs[j % 4][:], psy[j % 2][:], Tanh
                ).then_inc(tg, 1)

            # ---- end: settle DMA sems before cleanup's clear ----
            for s in lsem:
                nc.gpsimd.wait_ge(s, 16)
            for s in ssem:
                nc.gpsimd.wait_ge(s, 32)
            nc.all_engine_barrier()

    nc.compile()
    return nc


def _get_nc():
    if "nc" not in _CACHE:
        _CACHE["nc"] = _build_nc()
    return _CACHE["nc"]


def _prepare_in_maps(inputs) -> list[dict]:
    """Full inputs -> per-core in_maps (host does transpose + fp16 cast)."""
    x = np.asarray(inputs["x"], dtype=np.float32)
    hw = np.asarray(inputs["hidden_weights"], dtype=np.float32)
    cw = np.asarray(inputs["communication_weights"], dtype=np.float32)
    assert x.shape == (BATCH, D), x.shape

    wc7t = cw.T / np.float32(NORM)          # [64, 64]
    wdt = hw.T - wc7t                       # [64, 64]
    wpk = np.zeros((128, 384), dtype=np.float16)
    wpk[0:64, 0:64] = wdt                   # wd2 block-diagonal
    wpk[64:128, 64:128] = wdt
    wpk[0:64, 128:192] = wc7t               # wcs_lo
    wpk[0:64, 192:256] = wc7t
    wpk[64:128, 256:320] = wc7t             # wcs_hi
    wpk[64:128, 320:384] = wc7t

    s = x.reshape(BATCH, NAGENT, DA).sum(axis=1)
    x16 = x.astype(np.float16)
    s16 = s.astype(np.float16)

    in_maps = []
    for i in range(NCORES):
        rows = slice(i * SHARD, (i + 1) * SHARD)
        xt = x16[rows].T
        st = s16[rows].T
        x7 = np.empty((NGROUP, 128, GCOL), dtype=np.float16)
        x7[:, :, 0:XCOL] = (
            xt.reshape(NCHUNK, 128, NGROUP, R).transpose(2, 1, 0, 3)
            .reshape(NGROUP, 128, XCOL)
        )
        x7[:, :, XCOL:GCOL] = (
            st.reshape(DA, NGROUP, 2, 512).transpose(1, 2, 0, 3)
            .reshape(NGROUP, 128, SCOL)
        )
        in_maps.append({"x7": x7.reshape(NGROUP * 128, GCOL), "wpk": wpk})
    return in_maps


def _decode_out(res) -> np.ndarray:
    y = np.empty((BATCH, D), dtype=np.float32)
    inv = np.float32(1.0 / 127.0)
    for i, r in enumerate(res.results):
        y4 = r["y4"].reshape(NGROUP, 128, NCHUNK, R)
        yi = y4.transpose(0, 3, 2, 1).reshape(SHARD, D)
        y[i * SHARD:(i + 1) * SHARD] = yi
    y *= inv
    return y


def kernel(**inputs) -> np.ndarray:
    from concourse.bass_utils import run_bass_kernel_spmd

    nc = _get_nc()
    in_maps = _prepare_in_maps(inputs)
    res = run_bass_kernel_spmd(nc, in_maps, core_ids=list(range(NCORES)))
    return _decode_out(res)
